# revision 11
# baseline (speedup 1.0000x reference)
"""Differential multi-head attention Trainium2 Bass kernel.

Problem: B=4, N=1024, D=512, H=8 heads, DH=64. LAM=0.5.
  q = (x@Wq+bq)  -> [B,H,N,2*DH], halves q1,q2 (same for k)
  a_i = softmax(q_i@k_i^T / sqrt(DH)); attn = a1 - LAM*a2; out = attn@v

Sharding: 8 cores; core c handles batch b=c//2 and heads h0..h0+3 with
h0=(c%2)*4 (batch + head-group parallel). Weights column-sharded by head.

Per-core layout strategy (all compute on one NeuronCore):
  - x arrives transposed (xt [D,N]) so projections produce qT/kT [128,N]
    per head (dh on partitions) and v [N,DH] (keys on partitions) directly.
  - scores computed transposed: sT[k,q] = kT_half.T-block @ qT_half, K=64
    contraction; the two halves are packed on the PE array with
    tile_position row tiling.
  - exp via ScalarE ACTIVATE (scale=1/8 folded in); no max subtraction
    needed (inputs are small: |s/8| < ~4).
  - PV matmul with v augmented by a constant column (+1 for half1, -2 for
    half2) producing both u=e@v (transposed) and the softmax denominators
    in one accumulation chain. The -2 column folds -LAM/... into the
    reciprocal: out = u1/s1 + u2 * (1/(-2*s2)) = a1@v - 0.5*a2@v.
  - small [65,128] blocks transposed back via PE transpose; per-row
    normalization applied with DVE tensor_scalar ops; output DMA'd in
    natural [N, 256] layout.
"""
import sys

sys.path.insert(0, "/opt/trn_rl_repo")

from contextlib import ExitStack

import numpy as np

import concourse.bass as bass
import concourse.mybir as mybir
import concourse.tile as tile
from concourse import bacc, bass_utils
from concourse.masks import make_identity

F32 = mybir.dt.float32
F32R = mybir.dt.float32r
BF16 = mybir.dt.bfloat16

B, N, D, H = 4, 1024, 512, 8
DH = 64            # per-head dim for v and per q/k half
HPC = 4            # heads per core
LAM = 0.5
SCALE = 0.125      # 1/sqrt(DH)
NCORES = 8
CQ = HPC * 2 * DH  # 512 projection cols per core for q/k
CV = HPC * DH      # 256 projection cols per core for v
P = 128
NT = N // P        # 8 seq tiles
DC = D // P        # 4 contraction chunks
QW = 512           # query chunk width (fp32 matmul free-dim limit)
QC = N // QW       # 2 query chunks
AUG = DH + 1       # v columns + constant column

import os
PACK_SCORES = os.environ.get("K_PACK", "0") == "1"
STAGE = os.environ.get("K_STAGE", "full")  # proj | scores | pv | full
QKDT = F32R if os.environ.get("K_QKDT", "") == "f32r" else mybir.dt.bfloat16
XWDT = F32R if os.environ.get("K_XWDT", "") == "f32r" else mybir.dt.bfloat16


def build_nc(reps=1):
    nc = bacc.Bacc("TRN2", target_bir_lowering=False, debug=False,
                   num_devices=NCORES)
    d = {
        "xt": nc.dram_tensor("xt", [D, N], XWDT, kind="ExternalInput"),
        "wq": nc.dram_tensor("wq", [D, CQ], XWDT, kind="ExternalInput"),
        "wk": nc.dram_tensor("wk", [D, CQ], XWDT, kind="ExternalInput"),
        "wv": nc.dram_tensor("wv", [D, CV], XWDT, kind="ExternalInput"),
        "bq": nc.dram_tensor("bq", [P, HPC], F32, kind="ExternalInput"),
        "bk": nc.dram_tensor("bk", [P, HPC], F32, kind="ExternalInput"),
        "bvb": nc.dram_tensor("bvb", [P, CV], F32, kind="ExternalInput"),
        "o": nc.dram_tensor("o", [N, CV], F32, kind="ExternalOutput"),
    }
    with tile.TileContext(nc) as tc, ExitStack() as ctx:
        consts = ctx.enter_context(tc.tile_pool(name="consts", bufs=1))
        qk = ctx.enter_context(tc.tile_pool(name="qk", bufs=2))
        vaugp = ctx.enter_context(tc.tile_pool(name="vaugp", bufs=1))
        ep = ctx.enter_context(tc.tile_pool(name="ep", bufs=24))
        up = ctx.enter_context(tc.tile_pool(name="up", bufs=4))
        outp = ctx.enter_context(tc.tile_pool(name="outp", bufs=1))
        smallp = ctx.enter_context(tc.tile_pool(name="smallp", bufs=2))
        ps_proj = ctx.enter_context(
            tc.tile_pool(name="ps_proj", bufs=1, space="PSUM"))
        ps_score = ctx.enter_context(
            tc.tile_pool(name="ps_score", bufs=2, space="PSUM"))
        ps_pv = ctx.enter_context(
            tc.tile_pool(name="ps_pv", bufs=2, space="PSUM"))
        ps_tr = ctx.enter_context(
            tc.tile_pool(name="ps_tr", bufs=1, space="PSUM"))

        def body():
            # ---- input DMAs
            xt_sb, wq_sb, wk_sb, wv_sb = [], [], [], []
            for dc in range(DC):
                t = consts.tile([P, N], XWDT, tag=f"xt{dc}", name=f"xt{dc}")
                nc.sync.dma_start(t[:], d["xt"][dc * P:(dc + 1) * P, :])
                xt_sb.append(t)
                t = consts.tile([P, CQ], XWDT, tag=f"wq{dc}", name=f"wq{dc}")
                nc.sync.dma_start(t[:], d["wq"][dc * P:(dc + 1) * P, :])
                wq_sb.append(t)
                t = consts.tile([P, CQ], XWDT, tag=f"wk{dc}", name=f"wk{dc}")
                nc.sync.dma_start(t[:], d["wk"][dc * P:(dc + 1) * P, :])
                wk_sb.append(t)
            for dc in range(DC):
                t = consts.tile([P, CV], XWDT, tag=f"wv{dc}", name=f"wv{dc}")
                nc.sync.dma_start(t[:], d["wv"][dc * P:(dc + 1) * P, :])
                wv_sb.append(t)
            bq_sb = consts.tile([P, HPC], F32, tag="bq", name="bq")
            nc.sync.dma_start(bq_sb[:], d["bq"][:])
            bk_sb = consts.tile([P, HPC], F32, tag="bk", name="bk")
            nc.sync.dma_start(bk_sb[:], d["bk"][:])
            bvb_sb = consts.tile([P, CV], F32, tag="bvb", name="bvb")
            nc.sync.dma_start(bvb_sb[:], d["bvb"][:])
            ident = consts.tile([P, P], F32, tag="ident", name="ident")
            make_identity(nc, ident[:])

            def proj_qk(h, w_sb, b_sb, dest, pfx):
                # dest[c, n] = sum_d W[d, h*128+c] * xt[d, n] + b[c]
                for qc in range(QC):
                    ps = ps_proj.tile([P, QW], F32, tag="proj",
                                      name=f"ps_{pfx}{h}_{qc}")
                    for dc in range(DC):
                        nc.tensor.matmul(
                            ps[:],
                            w_sb[dc][:, h * P:(h + 1) * P],
                            xt_sb[dc][:, qc * QW:(qc + 1) * QW],
                            start=(dc == 0), stop=(dc == DC - 1))
                    nc.vector.tensor_scalar_add(
                        dest[:, qc * QW:(qc + 1) * QW], ps[:], b_sb[:, h:h + 1])

            def proj_head(h):
                qt = qk.tile([P, N], QKDT, tag="qt", name=f"qt{h}")
                kt_ = qk.tile([P, N], QKDT, tag="kt", name=f"kt{h}")
                proj_qk(h, wq_sb, bq_sb, qt, "q")
                proj_qk(h, wk_sb, bk_sb, kt_, "k")
                return qt, kt_

            qt0 = proj_head(0)

            # ---- v projection + augmentation (+1 / -2 constant columns)
            vaug1, vaug2 = [], []
            for nt in range(NT):
                ps = ps_proj.tile([P, CV], F32, tag="proj", name=f"ps_v{nt}")
                for dc in range(DC):
                    nc.tensor.matmul(
                        ps[:],
                        xt_sb[dc][:, nt * P:(nt + 1) * P],
                        wv_sb[dc][:],
                        start=(dc == 0), stop=(dc == DC - 1))
                t1 = vaugp.tile([P, HPC * AUG], BF16, tag=f"vaug1_{nt}",
                                name=f"vaug1_{nt}")
                t2 = vaugp.tile([P, HPC * AUG], BF16, tag=f"vaug2_{nt}",
                                name=f"vaug2_{nt}")
                t1v = t1[:].rearrange("p (h a) -> p h a", a=AUG)
                t2v = t2[:].rearrange("p (h a) -> p h a", a=AUG)
                psv = ps[:].rearrange("p (h a) -> p h a", a=DH)
                bvv = bvb_sb[:].rearrange("p (h a) -> p h a", a=DH)
                nc.vector.tensor_add(t1v[:, :, 0:DH], psv, bvv)
                nc.vector.tensor_add(t2v[:, :, 0:DH], psv, bvv)
                nc.vector.memset(t1v[:, :, DH:AUG], 1.0)
                nc.vector.memset(t2v[:, :, DH:AUG], -2.0)
                vaug1.append(t1)
                vaug2.append(t2)

            # ---- output staging tiles
            ostage = []
            for qt_i in range(NT):
                t = outp.tile([P, CV], F32, tag=f"ost{qt_i}", name=f"ost{qt_i}")
                ostage.append(t)

            def attn_head(h, qt, kt_):
                es = {}
                for half in range(2):
                    for kt in range(NT):
                        ps = ps_score.tile([P, N], F32, tag="score",
                                           name=f"ps_s{h}_{kt}_{half}")
                        lhsT = kt_[half * DH:(half + 1) * DH,
                                   kt * P:(kt + 1) * P]
                        for qc in range(QC):
                            rhs = qt[half * DH:(half + 1) * DH,
                                     qc * QW:(qc + 1) * QW]
                            kwargs = {}
                            if PACK_SCORES:
                                kwargs["tile_position"] = (half * DH, 0)
                            nc.tensor.matmul(
                                ps[:, qc * QW:(qc + 1) * QW], lhsT, rhs,
                                start=True, stop=True, **kwargs)
                        e = ep.tile([P, N], BF16, tag="e",
                                    name=f"e{h}_{kt}_{half}")
                        nc.scalar.activation(
                            e[:], ps[:], mybir.ActivationFunctionType.Exp,
                            scale=SCALE)
                        es[(half, kt)] = e
                return es

            def pv_head(h, es):
                us = []
                for half in range(2):
                    vv = vaug1 if half == 0 else vaug2
                    u = up.tile([AUG, N], F32, tag="u", name=f"u{h}_{half}")
                    for qc in range(QC):
                        ps = ps_pv.tile([AUG, QW], F32, tag="pv",
                                        name=f"ps_pv{h}_{half}_{qc}")
                        for kt in range(NT):
                            nc.tensor.matmul(
                                ps[:],
                                vv[kt][:, h * AUG:(h + 1) * AUG],
                                es[(half, kt)][:, qc * QW:(qc + 1) * QW]
                                ,
                                start=(kt == 0), stop=(kt == NT - 1))
                        nc.vector.tensor_copy(
                            u[:, qc * QW:(qc + 1) * QW], ps[:])
                    us.append(u)
                return us

            def finish_pair(hs, us_pair):
                for qt_i in range(NT):
                    tr = ps_tr.tile([P, 4 * AUG], F32, tag="tr",
                                    name=f"ps_tr{hs[0]}_{qt_i}")
                    for j, h in enumerate(hs):
                        u1, u2 = us_pair[h]
                        nc.tensor.transpose(
                            tr[:, (2 * j) * AUG:(2 * j + 1) * AUG],
                            u1[0:AUG, qt_i * P:(qt_i + 1) * P],
                            ident[0:AUG, 0:AUG])
                        nc.tensor.transpose(
                            tr[:, (2 * j + 1) * AUG:(2 * j + 2) * AUG],
                            u2[0:AUG, qt_i * P:(qt_i + 1) * P],
                            ident[0:AUG, 0:AUG])
                    rr = smallp.tile([P, 4], F32, tag="rr", name=f"rr_{hs[0]}{qt_i}")
                    trv = tr[:].rearrange("p (c a) -> p c a", a=AUG)
                    nc.vector.reciprocal(rr[:], trv[:, :, DH])
                    for j, h in enumerate(hs):
                        o1 = smallp.tile([P, DH], F32, tag="o1",
                                         name=f"o1_{h}{qt_i}")
                        o2 = smallp.tile([P, DH], F32, tag="o2",
                                         name=f"o2_{h}{qt_i}")
                        nc.vector.tensor_scalar_mul(
                            o1[:], tr[:, (2 * j) * AUG:(2 * j) * AUG + DH],
                            rr[:, 2 * j:2 * j + 1])
                        nc.vector.tensor_scalar_mul(
                            o2[:], tr[:, (2 * j + 1) * AUG:(2 * j + 1) * AUG + DH],
                            rr[:, 2 * j + 1:2 * j + 2])
                        nc.vector.tensor_add(
                            ostage[qt_i][:, h * DH:(h + 1) * DH], o1[:], o2[:])

            for pair in range(HPC // 2):
                us_pair = {}
                for h in (2 * pair, 2 * pair + 1):
                    qt, kt_ = qt0 if h == 0 else proj_head(h)
                    if STAGE == "proj":
                        nc.vector.tensor_copy(ostage[2 * h][:], qt[:, 0:CV])
                        nc.vector.tensor_copy(ostage[2 * h + 1][:], kt_[:, 0:CV])
                        continue
                    es = attn_head(h, qt, kt_)
                    if STAGE == "scores":
                        nc.vector.tensor_copy(ostage[2 * h][:], es[(0, 0)][:, 0:CV])
                        nc.vector.tensor_copy(ostage[2 * h + 1][:], es[(1, 7)][:, 0:CV])
                        continue
                    us_pair[h] = pv_head(h, es)
                if STAGE in ("proj", "scores"):
                    continue
                if STAGE == "pv":
                    for h in (2 * pair, 2 * pair + 1):
                        nc.vector.tensor_copy(ostage[2 * h][0:AUG, 0:CV],
                                              us_pair[h][0][:, 0:CV])
                        nc.vector.tensor_copy(ostage[2 * h + 1][0:AUG, 0:CV],
                                              us_pair[h][1][:, 0:CV])
                    continue
                finish_pair((2 * pair, 2 * pair + 1), us_pair)

            for qt_i in range(NT):
                nc.sync.dma_start(d["o"][qt_i * P:(qt_i + 1) * P, :],
                                  ostage[qt_i][:])

        if reps == 1:
            body()
        else:
            with tc.For_i(0, reps, 1,
                          hint_engines=(mybir.EngineType.PE,
                                        mybir.EngineType.DVE)):
                body()

    nc.compile()
    return nc


_NC_CACHE = {}


def get_nc(reps=1):
    if reps not in _NC_CACHE:
        _NC_CACHE[reps] = build_nc(reps)
    return _NC_CACHE[reps]


def shard_inputs(inputs):
    import ml_dtypes
    xw_np = (np.dtype(ml_dtypes.bfloat16)
             if XWDT == mybir.dt.bfloat16 else np.float32)
    x = np.asarray(inputs["x"], dtype=np.float32)
    Wq = np.asarray(inputs["Wq"], dtype=np.float32)
    bq = np.asarray(inputs["bq"], dtype=np.float32)
    Wk = np.asarray(inputs["Wk"], dtype=np.float32)
    bk = np.asarray(inputs["bk"], dtype=np.float32)
    Wv = np.asarray(inputs["Wv"], dtype=np.float32)
    bv = np.asarray(inputs["bv"], dtype=np.float32)
    in_maps = []
    for c in range(NCORES):
        b = c // 2
        h0 = (c % 2) * HPC
        cq0 = h0 * 2 * DH          # 0 or 512 in the q/k projection cols
        cv0 = h0 * DH              # 0 or 256 in the v cols
        in_maps.append({
            "xt": np.ascontiguousarray(x[b].T).astype(xw_np),
            "wq": np.ascontiguousarray(Wq[:, cq0:cq0 + CQ]).astype(xw_np),
            "wk": np.ascontiguousarray(Wk[:, cq0:cq0 + CQ]).astype(xw_np),
            "wv": np.ascontiguousarray(Wv[:, cv0:cv0 + CV]).astype(xw_np),
            "bq": np.ascontiguousarray(bq[cq0:cq0 + CQ].reshape(HPC, P).T),
            "bk": np.ascontiguousarray(bk[cq0:cq0 + CQ].reshape(HPC, P).T),
            "bvb": np.ascontiguousarray(
                np.broadcast_to(bv[cv0:cv0 + CV], (P, CV))),
        })
    return in_maps


def assemble_output(results):
    out = np.empty((B, N, D), dtype=np.float32)
    for c in range(NCORES):
        b = c // 2
        g = c % 2
        out[b, :, g * CV:(g + 1) * CV] = results[c]["o"]
    return out


def kernel(**inputs):
    nc = get_nc(1)
    in_maps = shard_inputs(inputs)
    res = bass_utils.run_bass_kernel_spmd(
        nc, in_maps, core_ids=list(range(NCORES)))
    return assemble_output(res.results)
